# revision 29
# baseline (speedup 1.0000x reference)
"""Multi-head causal self-attention (B=128, T=256, C=384, H=6, HS=64) for 8 TRN2 cores.

Strategy: pure data-parallel over batch (16 batch elements per core), weights
replicated, no collectives. Per batch-pair (2 elems, processed jointly):

  - x^T (pre-transposed on host, [C, 2T]) is the shared rhs/lhsT for projections
  - Q^T, K^T computed per head-pair as [128(2*d), 512(2*t)] PSUM tiles
  - V computed in natural [t, (h d)] layout
  - scores = K^T-slices.T @ Q^T with causal block-skipping, packed
    [s x tq(0:256) | s+128 x tq(128:256)] per head
  - softmax without max-subtraction (scores bounded for this distribution):
    exp on ACT, multiplicative causal mask on GpSimd.
  - row sums via PE matmul with a ones[128,64] stationary: the sums come out
    of the PE ALREADY BROADCAST across partitions, head A in partitions 0:64
    and head B in 64:128 (tile_position col-base 64). One DVE reciprocal per
    chain gives the normalizer tile nbr [128, 256] directly -- no separate
    broadcast matmul, no serialized [1,512] recip/cast round trip.
  - AV runs on the UNNORMALIZED exp'd scores (2 heads packed per PSUM tile,
    head B at partition base 64); normalization happens after AV as a single
    fused DVE multiply per chain: attT(bf16,SBUF) = av(PSUM) * nbr, which also
    replaces the separate PSUM->SBUF attT copy.
  - y = att^T.T @ Wp^T + bp, bias fused into the PSUM->SBUF move on DVE,
    y stored/DMA'd as bf16.

Scheduling, all aimed at keeping the PE HAM clock gate at 8/8 (2.4 GHz) by
never letting the PE idle:
  - software pipeline one pair deep: PE slot for pair N is
    [proj N][scores N (6 chains) interleaved with sums N-1][AV N-1][y N-1];
    every cross-engine dependency (ACT exp, GpSimd mask, DVE recip/norm) has
    microseconds of slack.
  - the score chains are SPREAD between sums thunks because ACT's exp drain
    (463ns/tile) is slower than the PE's score-tile fill (320ns/tile): a
    burst of 12 score tiles would exhaust any PSUM ring and stall the PE.
  - PSUM is bank-granular (8 banks): tag "st" gets a 3-bank ring, tag "work"
    (everything else) a 5-bank ring, so score-tile retirement (ACT) and
    work-tile retirement (DVE) never couple.
  - input DMAs are split per-k-chunk across 4 queues (sync/gpsimd/scalar/
    vector) so the first projection matmul can start as soon as the engine
    preambles end, instead of serializing 300KB+ behind one queue.

Matmul operands in bf16 (fp32 PSUM accumulation), softmax stats in fp32.
"""

import numpy as np
import ml_dtypes
from contextlib import ExitStack

import concourse.bass as bass
import concourse.bacc as bacc
import concourse.mybir as mybir
import concourse.tile as tile
from concourse.bass_utils import run_bass_kernel_spmd

B, T, C, H, HS = 128, 256, 384, 6, 64
NCORES = 8
BPC = B // NCORES  # batch elements per core

F32 = mybir.dt.float32
DT = mybir.dt.bfloat16
F8 = mybir.dt.float8e4
NPDT = ml_dtypes.bfloat16
NPF8 = ml_dtypes.float8_e4m3
DR = mybir.MatmulPerfMode.DoubleRow

EXP = mybir.ActivationFunctionType.Exp

# fp8 weight pre-scale: Wq/Wk values (~2.5e-3 std) sit in e4m3's subnormal
# range, so both are scaled by FP8S on the host; the combined 1/FP8S^2 comes
# out for free via the exp activation's scalar `scale`.
FP8S = 64.0


def build(n_batch: int = BPC) -> bass.Bass:
    assert n_batch % 2 == 0
    npair = n_batch // 2
    nc = bacc.Bacc("TRN2", target_bir_lowering=False, debug=False)

    xT = nc.dram_tensor("xT", [npair, 3, 128, 2 * T], DT, kind="ExternalInput").ap()
    xT8 = nc.dram_tensor("xT8", [npair, 128, 2, 2 * T], F8, kind="ExternalInput").ap()
    wq8 = nc.dram_tensor("wq8", [128, 2, 3, 128], F8, kind="ExternalInput").ap()
    wk8 = nc.dram_tensor("wk8", [128, 2, 3, 128], F8, kind="ExternalInput").ap()
    wq2 = nc.dram_tensor("wq2", [128, 3, 128], DT, kind="ExternalInput").ap()
    wk2 = nc.dram_tensor("wk2", [128, 3, 128], DT, kind="ExternalInput").ap()
    wv = nc.dram_tensor("wv", [128, 3, C], DT, kind="ExternalInput").ap()
    wp = nc.dram_tensor("wp", [128, 3, C], DT, kind="ExternalInput").ap()
    msk = nc.dram_tensor("msk", [128, 128], DT, kind="ExternalInput").ap()
    bb = nc.dram_tensor("bb", [128, C], F32, kind="ExternalInput").ap()
    y = nc.dram_tensor("y", [n_batch, T, C], DT, kind="ExternalOutput").ap()

    with tile.TileContext(nc) as tc, ExitStack() as ctx:
        const = ctx.enter_context(tc.tile_pool(name="const", bufs=1))
        sb = ctx.enter_context(tc.tile_pool(name="sb", bufs=2))
        psa = ctx.enter_context(tc.tile_pool(name="psa", bufs=5, space="PSUM"))

        # --- weight / first-x DMAs, split per k-chunk across 3 queues so
        # the pieces the first projection matmuls need land first ---
        wq8_t = const.tile([128, 2, 3, 128], F8)
        wk8_t = const.tile([128, 2, 3, 128], F8)
        wq2_t = const.tile([128, 3, 128], DT)
        wk2_t = const.tile([128, 3, 128], DT)
        wv_t = const.tile([128, 3, C], DT)
        wp_t = const.tile([128, 3, C], DT)
        msk_t = const.tile([128, 128], DT)
        bb_t = const.tile([128, C], F32)

        xts = {}

        def xt_dma(pair):
            xt = sb.tile([128, 3, 2 * T], DT, tag="xt", bufs=4, name=f"xt_{pair}")
            xt8 = sb.tile([128, 2, 2 * T], F8, tag="xt8", bufs=4,
                          name=f"xt8_{pair}")
            nc.sync.dma_start(out=xt[:, 0, :], in_=xT[pair, 0])
            nc.gpsimd.dma_start(out=xt[:, 1, :], in_=xT[pair, 1])
            nc.scalar.dma_start(out=xt[:, 2, :], in_=xT[pair, 2])
            nc.sync.dma_start(out=xt8, in_=xT8[pair])
            xts[pair] = (xt, xt8)

        # HAM warm-up + early-queue boilerplate on gpsimd (earliest preamble)
        warm_in = const.tile([128, 512], DT)
        nc.gpsimd.memset(warm_in, 0.0)
        ones_sq = const.tile([128, 64], DT)
        nc.gpsimd.memset(ones_sq, 1.0)

        # pair 0 / 1 x prefetch interleaved with the weight chunks; ordered
        # per-queue so arrival matches first-use order of the matmul stream
        xt0 = sb.tile([128, 3, 2 * T], DT, tag="xt", bufs=4, name="xt_0")
        xt08 = sb.tile([128, 2, 2 * T], F8, tag="xt8", bufs=4, name="xt8_0")
        xt1 = sb.tile([128, 3, 2 * T], DT, tag="xt", bufs=4, name="xt_1")
        xt18 = sb.tile([128, 2, 2 * T], F8, tag="xt8", bufs=4, name="xt8_1")
        xts[0] = (xt0, xt08)
        xts[1] = (xt1, xt18)
        nc.scalar.dma_start(out=wq8_t, in_=wq8)
        nc.gpsimd.dma_start(out=wk8_t, in_=wk8)
        nc.sync.dma_start(out=xt08, in_=xT8[0])
        nc.scalar.dma_start(out=xt0[:, 2, :], in_=xT[0, 2])
        nc.gpsimd.dma_start(out=wq2_t, in_=wq2)
        nc.scalar.dma_start(out=wk2_t, in_=wk2)
        nc.sync.dma_start(out=xt0[:, 0, :], in_=xT[0, 0])
        nc.gpsimd.dma_start(out=xt0[:, 1, :], in_=xT[0, 1])
        nc.gpsimd.dma_start(out=wv_t[:, 0], in_=wv[:, 0])
        nc.scalar.dma_start(out=wv_t[:, 2], in_=wv[:, 2])
        nc.sync.dma_start(out=wv_t[:, 1], in_=wv[:, 1])
        nc.gpsimd.dma_start(out=msk_t, in_=msk)
        nc.scalar.dma_start(out=xt1[:, 2, :], in_=xT[1, 2])
        nc.gpsimd.dma_start(out=xt1[:, 1, :], in_=xT[1, 1])
        nc.sync.dma_start(out=xt1[:, 0, :], in_=xT[1, 0])
        nc.sync.dma_start(out=xt18, in_=xT8[1])
        nc.gpsimd.dma_start(out=bb_t, in_=bb)
        nc.gpsimd.dma_start(out=wp_t, in_=wp)

        warm_ps = psa.tile([128, 512], F32, tag="work")
        for _ in range(10):
            nc.tensor.matmul(
                warm_ps, lhsT=warm_in[:, 0:128], rhs=warm_in,
                start=True, stop=True,
            )

        def filler(n):
            # pair-0 only: dummy matmuls bridging initial DMA-wait gaps so
            # the HAM busy-window keeps accumulating and un-throttles early
            fps = psa.tile([128, 384], F32, tag="st", bufs=3, name="filler")
            for _ in range(n):
                nc.tensor.matmul(
                    fps, lhsT=warm_in[:, 0:128], rhs=warm_in[:, 0:384],
                    start=True, stop=True,
                )

        def stage_proj(pair):
            """QKV projections for `pair` (qt/kt copies: qt->ACT, kt/v->DVE).

            Q/K contract k-chunks 0+1 in one fp8 DoubleRow matmul (weights
            pre-scaled by FP8S on host) and k-chunk 2 in bf16.
            """
            xt, xt8 = xts.pop(pair)
            qt = sb.tile([128, 3, 2 * T], DT, tag="qt", bufs=3)
            kt = sb.tile([128, 3, 2 * T], DT, tag="kt", bufs=3)
            for p in range(3):
                qt_ps = psa.tile([128, 2 * T], F32, tag="work")
                nc.tensor.matmul(
                    qt_ps,
                    lhsT=wq8_t[:, :, p, :],
                    rhs=xt8,
                    start=True,
                    stop=False,
                    perf_mode=DR,
                )
                nc.tensor.matmul(
                    qt_ps,
                    lhsT=wq2_t[:, p, :],
                    rhs=xt[:, 2, :],
                    start=False,
                    stop=True,
                )
                nc.scalar.copy(out=qt[:, p, :], in_=qt_ps)
                kt_ps = psa.tile([128, 2 * T], F32, tag="work")
                nc.tensor.matmul(
                    kt_ps,
                    lhsT=wk8_t[:, :, p, :],
                    rhs=xt8,
                    start=True,
                    stop=False,
                    perf_mode=DR,
                )
                nc.tensor.matmul(
                    kt_ps,
                    lhsT=wk2_t[:, p, :],
                    rhs=xt[:, 2, :],
                    start=False,
                    stop=True,
                )
                nc.vector.tensor_copy(out=kt[:, p, :], in_=kt_ps)

            def do_v():
                vs = []
                for bi in range(2):
                    v = sb.tile([128, 2, C], DT, tag="v", bufs=6,
                                name=f"v_{pair}_{bi}")
                    for m in range(2):
                        v_ps = psa.tile([128, C], F32, tag="work")
                        for k in range(3):
                            nc.tensor.matmul(
                                v_ps,
                                lhsT=xt[:, k, bi * T + m * 128 : bi * T + (m + 1) * 128],
                                rhs=wv_t[:, k, :],
                                start=(k == 0),
                                stop=(k == 2),
                            )
                        nc.vector.tensor_copy(out=v[:, m, :], in_=v_ps)
                    vs.append(v)
                return vs
            return do_v, qt, kt

        def mk_score_chain(pair, qt, kt, pexs, bi, pr):
            """One (bi, pr) head-pair: 4 score MMs + 2 exps + 1 mask."""
            def th():
                pex = sb.tile(
                    [128, 2, 384], DT, tag="pex", bufs=14,
                    name=f"pex_{pair}_{bi}_{pr}",
                )
                for two in range(2):
                    lo = two * 64
                    qh = qt[lo : lo + 64, pr, bi * T : (bi + 1) * T]
                    kh = kt[lo : lo + 64, pr, bi * T : (bi + 1) * T]
                    st = psa.tile([128, 384], F32, tag="st", bufs=3)
                    nc.tensor.matmul(
                        st[:, 0:256],
                        lhsT=kh[:, 0:128],
                        rhs=qh,
                        start=True,
                        stop=True,
                    )
                    nc.tensor.matmul(
                        st[:, 256:384],
                        lhsT=kh[:, 128:256],
                        rhs=qh[:, 128:256],
                        start=True,
                        stop=True,
                    )
                    nc.scalar.activation(out=pex[:, two, :], in_=st, func=EXP,
                                         scale=1.0 / (FP8S * FP8S))
                # multiplicative causal mask (keep tq >= s) on the four
                # triangular blocks of the packed 2-head pex in ONE op:
                # dims [p][head][block in {0:128, 256:384}][j]
                pexv = bass.AP(
                    tensor=pex.tensor,
                    offset=pex.offset,
                    ap=[pex.ap[0], pex.ap[1], [256, 2], [1, 128]],
                )
                mskb = bass.AP(
                    tensor=msk_t.tensor,
                    offset=msk_t.offset,
                    ap=[msk_t.ap[0], [0, 2], [0, 2], msk_t.ap[1]],
                )
                nc.gpsimd.tensor_mul(out=pexv, in0=pexv, in1=mskb)
                pexs[(bi, pr)] = pex
            return th

        def stage_b_thunks(pair, vs, pexs):
            """Deferred sums + AV + normalize + output projection for `pair`.

            Returns (sums_thunks[6], av_thunks[6], y_thunks[2]); issued during
            the NEXT pair's slot so all cross-engine deps have slack.
            """
            attTs = {}
            nbrs = {}

            def mk_sums(bi, pr):
                def th():
                    pex = pexs[(bi, pr)]
                    nb = psa.tile([128, 256], F32, tag="work",
                                  name=f"nb_{pair}_{bi}_{pr}")
                    for two in range(2):
                        lo = two * 64
                        nc.tensor.matmul(
                            nb[lo : lo + 64, 0:256],
                            lhsT=ones_sq,
                            rhs=pex[:, two, 0:256],
                            start=True,
                            stop=False,
                            skip_group_check=True,
                        )
                        nc.tensor.matmul(
                            nb[lo : lo + 64, 128:256],
                            lhsT=ones_sq,
                            rhs=pex[:, two, 256:384],
                            start=False,
                            stop=True,
                            skip_group_check=True,
                        )
                    nbr = sb.tile([128, 256], F32, tag="nbr", bufs=8,
                                  name=f"nbr_{pair}_{bi}_{pr}")
                    nc.vector.reciprocal_approx_fast(out=nbr, in_=nb)
                    nbrs[(bi, pr)] = nbr
                return th

            def mk_av(bi, pr):
                def th():
                    if pr == 0:
                        attTs[bi] = sb.tile(
                            [128, 3, 256], DT, tag="attT", bufs=4,
                            name=f"attT_{pair}_{bi}",
                        )
                    attT = attTs[bi]
                    pex = pexs[(bi, pr)]
                    v = vs[bi]
                    av = psa.tile([128, 256], F32, tag="work",
                                  name=f"av_{pair}_{bi}_{pr}")
                    for two in range(2):
                        h = 2 * pr + two
                        lo = two * 64
                        hs = slice(h * 64, h * 64 + 64)
                        nc.tensor.matmul(
                            av[lo : lo + 64, 0:256],
                            lhsT=v[:, 0, hs],
                            rhs=pex[:, two, 0:256],
                            start=True,
                            stop=False,
                            skip_group_check=True,
                        )
                        nc.tensor.matmul(
                            av[lo : lo + 64, 128:256],
                            lhsT=v[:, 1, hs],
                            rhs=pex[:, two, 256:384],
                            start=False,
                            stop=True,
                            skip_group_check=True,
                        )
                    # fused normalize + PSUM->SBUF move
                    nc.vector.tensor_mul(out=attT[:, pr, :], in0=av,
                                         in1=nbrs[(bi, pr)])
                return th

            def mk_y(bi):
                def th():
                    attT = attTs[bi]
                    for m in range(2):
                        y_ps = psa.tile([128, C], F32, tag="work",
                                        name=f"y_{pair}_{bi}_{m}")
                        for k in range(3):
                            nc.tensor.matmul(
                                y_ps,
                                lhsT=attT[:, k, bass.ts(m, 128)],
                                rhs=wp_t[:, k, :],
                                start=(k == 0),
                                stop=(k == 2),
                            )
                        ysb = sb.tile([128, C], DT, tag="ysb", bufs=4,
                                      name=f"ysb_{pair}_{bi}_{m}")
                        nc.vector.tensor_add(out=ysb, in0=y_ps, in1=bb_t)
                        nc.sync.dma_start(
                            out=y[2 * pair + bi, bass.ts(m, 128), :], in_=ysb
                        )
                return th

            sums = [mk_sums(bi, pr) for bi in range(2) for pr in range(3)]
            avs = [mk_av(bi, pr) for bi in range(2) for pr in range(3)]
            ys = [mk_y(bi) for bi in range(2)]
            return sums, avs, ys

        prev_b = None
        for pair in range(npair):
            if pair + 2 < npair:
                xt_dma(pair + 2)
            vthunk, qt, kt = stage_proj(pair)
            pexs = {}
            score_thunks = [
                mk_score_chain(pair, qt, kt, pexs, bi, pr)
                for bi in range(2) for pr in range(3)
            ]
            # interleave: scores of `pair` spread between the previous pair's
            # sums so ACT's exp drain keeps pace with st-ring allocation
            if prev_b is None:
                # cold start: the first two score chains need only qt/kt,
                # so they fill the PE while the wv DMA is still in flight
                score_thunks[0]()
                score_thunks[1]()
                vs = vthunk()
                cur_b = stage_b_thunks(pair, vs, pexs)
                for th in score_thunks[2:]:
                    th()
            else:
                vs = vthunk()
                cur_b = stage_b_thunks(pair, vs, pexs)
                sums, avs, ys = prev_b
                for i in range(6):
                    score_thunks[i]()
                    sums[i]()
                if pair == npair - 1:
                    # final slot: pull the last pair's own sums/AV/y into the
                    # stream so the tail's DVE chain overlaps PE work
                    sums7, avs7, ys7 = cur_b
                    for th in avs:
                        th()
                    for i in range(3):
                        sums7[i]()
                    ys[0]()
                    for i in range(3, 6):
                        sums7[i]()
                    ys[1]()
                    for i in range(3):
                        avs7[i]()
                    ys7[0]()
                    for i in range(3, 6):
                        avs7[i]()
                    ys7[1]()
                    cur_b = None
                else:
                    for th in avs:
                        th()
                    for th in ys:
                        th()
            prev_b = cur_b

        if prev_b is not None:
            sums, avs, ys = prev_b
            for bi in range(2):
                for i in range(3 * bi, 3 * bi + 3):
                    sums[i]()
                for i in range(3 * bi, 3 * bi + 3):
                    avs[i]()
                ys[bi]()
    nc.compile()
    return nc


def pack_inputs(x, Wq, Wk, Wv, Wp, bp):
    """Host-side packing. Returns (common weight map, per-core xT shards)."""
    from einops import rearrange

    x = np.asarray(x, np.float32)
    Wq = np.asarray(Wq, np.float32)
    Wk = np.asarray(Wk, np.float32)
    Wv = np.asarray(Wv, np.float32)
    Wp = np.asarray(Wp, np.float32)
    bp = np.asarray(bp, np.float32)

    scale = 1.0 / np.sqrt(np.float32(HS))
    wq_h = rearrange(Wq * scale, "(p two) (k c) d -> c k p (two d)", two=2, k=3)
    wk_h = rearrange(Wk, "(p two) (k c) d -> c k p (two d)", two=2, k=3)
    wv_h = rearrange(Wv, "h (k c) d -> c k (h d)", k=3)
    wp_h = rearrange(Wp, "c2 (k c1) -> c1 k c2", k=3)

    # multiplicative causal mask for a diagonal [128,128] block of the
    # TRANSPOSED scores st[s, tq]: keep tq >= s, i.e. 1 if j >= i else 0
    msk_h = np.triu(np.ones((128, 128), np.float32))
    bb_h = np.tile(bp[None, :], (128, 1)).astype(np.float32)

    common = {
        "wq8": np.ascontiguousarray(wq_h[:, 0:2] * FP8S).astype(NPF8),
        "wk8": np.ascontiguousarray(wk_h[:, 0:2] * FP8S).astype(NPF8),
        "wq2": np.ascontiguousarray(wq_h[:, 2] * FP8S).astype(NPDT),
        "wk2": np.ascontiguousarray(wk_h[:, 2] * FP8S).astype(NPDT),
        "wv": np.ascontiguousarray(wv_h).astype(NPDT),
        "wp": np.ascontiguousarray(wp_h).astype(NPDT),
        "msk": msk_h.astype(NPDT),
        "bb": bb_h,
    }
    shards = []
    for c in range(NCORES):
        xs = x[c * BPC : (c + 1) * BPC]  # [BPC, T, C]
        # paired layout: [pair, kc, c_local, b'*T + t]
        xp = xs.reshape(BPC // 2, 2, T, C).transpose(0, 3, 1, 2)  # [pair, C, 2, T]
        xTs = xp.reshape(BPC // 2, 3, 128, 2 * T)
        shards.append((
            np.ascontiguousarray(xTs).astype(NPDT),
            np.ascontiguousarray(xTs[:, 0:2].transpose(0, 2, 1, 3)).astype(NPF8),
        ))
    return common, shards


_NC_CACHE = {}


def _get_nc(n_batch: int = BPC) -> bass.Bass:
    if n_batch not in _NC_CACHE:
        _NC_CACHE[n_batch] = build(n_batch)
    return _NC_CACHE[n_batch]


def kernel(x, Wq, Wk, Wv, Wp, bp):
    common, shards = pack_inputs(x, Wq, Wk, Wv, Wp, bp)
    nc = _get_nc()
    in_maps = [
        {**common, "xT": shards[c][0], "xT8": shards[c][1]}
        for c in range(NCORES)
    ]
    res = run_bass_kernel_spmd(nc, in_maps, list(range(NCORES))).results
    y = np.concatenate([res[c]["y"] for c in range(NCORES)], axis=0)
    return np.ascontiguousarray(y.astype(np.float32))


# revision 30
# speedup vs baseline: 1.0051x; 1.0051x over previous
"""Multi-head causal self-attention (B=128, T=256, C=384, H=6, HS=64) for 8 TRN2 cores.

Strategy: pure data-parallel over batch (16 batch elements per core), weights
replicated, no collectives. Per batch-pair (2 elems, processed jointly):

  - x^T (pre-transposed on host, [C, 2T]) is the shared rhs/lhsT for projections
  - Q^T, K^T computed per head-pair as [128(2*d), 512(2*t)] PSUM tiles
  - V computed in natural [t, (h d)] layout
  - scores = K^T-slices.T @ Q^T with causal block-skipping, packed
    [s x tq(0:256) | s+128 x tq(128:256)] per head
  - softmax without max-subtraction (scores bounded for this distribution):
    exp on ACT, multiplicative causal mask on GpSimd.
  - row sums via PE matmul with a ones[128,64] stationary: the sums come out
    of the PE ALREADY BROADCAST across partitions, head A in partitions 0:64
    and head B in 64:128 (tile_position col-base 64). One DVE reciprocal per
    chain gives the normalizer tile nbr [128, 256] directly -- no separate
    broadcast matmul, no serialized [1,512] recip/cast round trip.
  - AV runs on the UNNORMALIZED exp'd scores (2 heads packed per PSUM tile,
    head B at partition base 64); normalization happens after AV as a single
    fused DVE multiply per chain: attT(bf16,SBUF) = av(PSUM) * nbr, which also
    replaces the separate PSUM->SBUF attT copy.
  - y = att^T.T @ Wp^T + bp, bias fused into the PSUM->SBUF move on DVE,
    y stored/DMA'd as bf16.

Scheduling, all aimed at keeping the PE HAM clock gate at 8/8 (2.4 GHz) by
never letting the PE idle:
  - software pipeline one pair deep: PE slot for pair N is
    [proj N][scores N (6 chains) interleaved with sums N-1][AV N-1][y N-1];
    every cross-engine dependency (ACT exp, GpSimd mask, DVE recip/norm) has
    microseconds of slack.
  - the score chains are SPREAD between sums thunks because ACT's exp drain
    (463ns/tile) is slower than the PE's score-tile fill (320ns/tile): a
    burst of 12 score tiles would exhaust any PSUM ring and stall the PE.
  - PSUM is bank-granular (8 banks): tag "st" gets a 3-bank ring, tag "work"
    (everything else) a 5-bank ring, so score-tile retirement (ACT) and
    work-tile retirement (DVE) never couple.
  - input DMAs are split per-k-chunk across 4 queues (sync/gpsimd/scalar/
    vector) so the first projection matmul can start as soon as the engine
    preambles end, instead of serializing 300KB+ behind one queue.

Matmul operands in bf16 (fp32 PSUM accumulation), softmax stats in fp32.
"""

import numpy as np
import ml_dtypes
from contextlib import ExitStack

import concourse.bass as bass
import concourse.bacc as bacc
import concourse.mybir as mybir
import concourse.tile as tile
from concourse.bass_utils import run_bass_kernel_spmd

B, T, C, H, HS = 128, 256, 384, 6, 64
NCORES = 8
BPC = B // NCORES  # batch elements per core

F32 = mybir.dt.float32
DT = mybir.dt.bfloat16
F8 = mybir.dt.float8e4
NPDT = ml_dtypes.bfloat16
NPF8 = ml_dtypes.float8_e4m3
DR = mybir.MatmulPerfMode.DoubleRow

EXP = mybir.ActivationFunctionType.Exp

# fp8 weight pre-scale: Wq/Wk values (~2.5e-3 std) sit in e4m3's subnormal
# range, so both are scaled by FP8S on the host; the combined 1/FP8S^2 comes
# out for free via the exp activation's scalar `scale`.
FP8S = 64.0


def build(n_batch: int = BPC) -> bass.Bass:
    assert n_batch % 2 == 0
    npair = n_batch // 2
    nc = bacc.Bacc("TRN2", target_bir_lowering=False, debug=False)

    xT = nc.dram_tensor("xT", [npair, 3, 128, 2 * T], DT, kind="ExternalInput").ap()
    xT8 = nc.dram_tensor("xT8", [npair, 128, 2, 2 * T], F8, kind="ExternalInput").ap()
    wq8 = nc.dram_tensor("wq8", [128, 2, 3, 128], F8, kind="ExternalInput").ap()
    wk8 = nc.dram_tensor("wk8", [128, 2, 3, 128], F8, kind="ExternalInput").ap()
    wq2 = nc.dram_tensor("wq2", [128, 3, 128], DT, kind="ExternalInput").ap()
    wk2 = nc.dram_tensor("wk2", [128, 3, 128], DT, kind="ExternalInput").ap()
    wv = nc.dram_tensor("wv", [128, 3, C], DT, kind="ExternalInput").ap()
    wp = nc.dram_tensor("wp", [128, 3, C], DT, kind="ExternalInput").ap()
    msk = nc.dram_tensor("msk", [128, 128], DT, kind="ExternalInput").ap()
    bb = nc.dram_tensor("bb", [128, C], F32, kind="ExternalInput").ap()
    y = nc.dram_tensor("y", [n_batch, T, C], DT, kind="ExternalOutput").ap()

    with tile.TileContext(nc) as tc, ExitStack() as ctx:
        const = ctx.enter_context(tc.tile_pool(name="const", bufs=1))
        sb = ctx.enter_context(tc.tile_pool(name="sb", bufs=2))
        psa = ctx.enter_context(tc.tile_pool(name="psa", bufs=5, space="PSUM"))

        # --- weight / first-x DMAs, split per k-chunk across 3 queues so
        # the pieces the first projection matmuls need land first ---
        wq8_t = const.tile([128, 2, 3, 128], F8)
        wk8_t = const.tile([128, 2, 3, 128], F8)
        wq2_t = const.tile([128, 3, 128], DT)
        wk2_t = const.tile([128, 3, 128], DT)
        wv_t = const.tile([128, 3, C], DT)
        wp_t = const.tile([128, 3, C], DT)
        msk_t = const.tile([128, 128], DT)
        bb_t = const.tile([128, C], F32)

        xts = {}

        def xt_dma(pair):
            xt = sb.tile([128, 3, 2 * T], DT, tag="xt", bufs=4, name=f"xt_{pair}")
            xt8 = sb.tile([128, 2, 2 * T], F8, tag="xt8", bufs=4,
                          name=f"xt8_{pair}")
            nc.sync.dma_start(out=xt[:, 0, :], in_=xT[pair, 0])
            nc.gpsimd.dma_start(out=xt[:, 1, :], in_=xT[pair, 1])
            nc.scalar.dma_start(out=xt[:, 2, :], in_=xT[pair, 2])
            nc.sync.dma_start(out=xt8, in_=xT8[pair])
            xts[pair] = (xt, xt8)

        # HAM warm-up + early-queue boilerplate on gpsimd (earliest preamble)
        warm_in = const.tile([128, 512], DT)
        nc.gpsimd.memset(warm_in, 0.0)
        ones_sq = const.tile([128, 64], DT)
        nc.gpsimd.memset(ones_sq, 1.0)

        # pair 0 / 1 x prefetch interleaved with the weight chunks; ordered
        # per-queue so arrival matches first-use order of the matmul stream
        xt0 = sb.tile([128, 3, 2 * T], DT, tag="xt", bufs=4, name="xt_0")
        xt08 = sb.tile([128, 2, 2 * T], F8, tag="xt8", bufs=4, name="xt8_0")
        xt1 = sb.tile([128, 3, 2 * T], DT, tag="xt", bufs=4, name="xt_1")
        xt18 = sb.tile([128, 2, 2 * T], F8, tag="xt8", bufs=4, name="xt8_1")
        xts[0] = (xt0, xt08)
        xts[1] = (xt1, xt18)
        nc.scalar.dma_start(out=wq8_t, in_=wq8)
        nc.gpsimd.dma_start(out=wk8_t, in_=wk8)
        nc.sync.dma_start(out=xt08, in_=xT8[0])
        nc.scalar.dma_start(out=xt0[:, 2, :], in_=xT[0, 2])
        nc.gpsimd.dma_start(out=wq2_t, in_=wq2)
        nc.scalar.dma_start(out=wk2_t, in_=wk2)
        nc.sync.dma_start(out=xt0[:, 0, :], in_=xT[0, 0])
        nc.gpsimd.dma_start(out=xt0[:, 1, :], in_=xT[0, 1])
        nc.gpsimd.dma_start(out=wv_t[:, 0], in_=wv[:, 0])
        nc.scalar.dma_start(out=wv_t[:, 2], in_=wv[:, 2])
        nc.sync.dma_start(out=wv_t[:, 1], in_=wv[:, 1])
        nc.gpsimd.dma_start(out=msk_t, in_=msk)
        nc.scalar.dma_start(out=xt1[:, 2, :], in_=xT[1, 2])
        nc.gpsimd.dma_start(out=xt1[:, 1, :], in_=xT[1, 1])
        nc.sync.dma_start(out=xt1[:, 0, :], in_=xT[1, 0])
        nc.sync.dma_start(out=xt18, in_=xT8[1])
        nc.gpsimd.dma_start(out=bb_t, in_=bb)
        nc.gpsimd.dma_start(out=wp_t, in_=wp)

        warm_ps = psa.tile([128, 512], F32, tag="work")
        for _ in range(10):
            nc.tensor.matmul(
                warm_ps, lhsT=warm_in[:, 0:128], rhs=warm_in,
                start=True, stop=True,
            )

        def filler(n):
            # pair-0 only: dummy matmuls bridging initial DMA-wait gaps so
            # the HAM busy-window keeps accumulating and un-throttles early
            fps = psa.tile([128, 384], F32, tag="st", bufs=3, name="filler")
            for _ in range(n):
                nc.tensor.matmul(
                    fps, lhsT=warm_in[:, 0:128], rhs=warm_in[:, 0:384],
                    start=True, stop=True,
                )

        def stage_proj(pair):
            """QKV projections for `pair` (qt/kt copies: qt->ACT, kt/v->DVE).

            Q/K contract k-chunks 0+1 in one fp8 DoubleRow matmul (weights
            pre-scaled by FP8S on host) and k-chunk 2 in bf16.
            """
            xt, xt8 = xts.pop(pair)
            qt = sb.tile([128, 3, 2 * T], DT, tag="qt", bufs=3)
            kt = sb.tile([128, 3, 2 * T], DT, tag="kt", bufs=3)
            for p in range(3):
                qt_ps = psa.tile([128, 2 * T], F32, tag="work")
                nc.tensor.matmul(
                    qt_ps,
                    lhsT=wq8_t[:, :, p, :],
                    rhs=xt8,
                    start=True,
                    stop=False,
                    perf_mode=DR,
                )
                nc.tensor.matmul(
                    qt_ps,
                    lhsT=wq2_t[:, p, :],
                    rhs=xt[:, 2, :],
                    start=False,
                    stop=True,
                )
                nc.scalar.copy(out=qt[:, p, :], in_=qt_ps)
                kt_ps = psa.tile([128, 2 * T], F32, tag="work")
                nc.tensor.matmul(
                    kt_ps,
                    lhsT=wk8_t[:, :, p, :],
                    rhs=xt8,
                    start=True,
                    stop=False,
                    perf_mode=DR,
                )
                nc.tensor.matmul(
                    kt_ps,
                    lhsT=wk2_t[:, p, :],
                    rhs=xt[:, 2, :],
                    start=False,
                    stop=True,
                )
                nc.vector.tensor_copy(out=kt[:, p, :], in_=kt_ps)

            vs = []
            for bi in range(2):
                v = sb.tile([128, 2, C], DT, tag="v", bufs=6, name=f"v_{pair}_{bi}")
                for m in range(2):
                    v_ps = psa.tile([128, C], F32, tag="work")
                    for k in range(3):
                        nc.tensor.matmul(
                            v_ps,
                            lhsT=xt[:, k, bi * T + m * 128 : bi * T + (m + 1) * 128],
                            rhs=wv_t[:, k, :],
                            start=(k == 0),
                            stop=(k == 2),
                        )
                    nc.vector.tensor_copy(out=v[:, m, :], in_=v_ps)
                vs.append(v)
            return vs, qt, kt

        def mk_score_chain(pair, qt, kt, pexs, bi, pr):
            """One (bi, pr) head-pair: 4 score MMs + 2 exps + 1 mask."""
            def th():
                pex = sb.tile(
                    [128, 2, 384], DT, tag="pex", bufs=14,
                    name=f"pex_{pair}_{bi}_{pr}",
                )
                for two in range(2):
                    lo = two * 64
                    qh = qt[lo : lo + 64, pr, bi * T : (bi + 1) * T]
                    kh = kt[lo : lo + 64, pr, bi * T : (bi + 1) * T]
                    st = psa.tile([128, 384], F32, tag="st", bufs=3)
                    nc.tensor.matmul(
                        st[:, 0:256],
                        lhsT=kh[:, 0:128],
                        rhs=qh,
                        start=True,
                        stop=True,
                    )
                    nc.tensor.matmul(
                        st[:, 256:384],
                        lhsT=kh[:, 128:256],
                        rhs=qh[:, 128:256],
                        start=True,
                        stop=True,
                    )
                    nc.scalar.activation(out=pex[:, two, :], in_=st, func=EXP,
                                         scale=1.0 / (FP8S * FP8S))
                # multiplicative causal mask (keep tq >= s) on the four
                # triangular blocks of the packed 2-head pex in ONE op:
                # dims [p][head][block in {0:128, 256:384}][j]
                pexv = bass.AP(
                    tensor=pex.tensor,
                    offset=pex.offset,
                    ap=[pex.ap[0], pex.ap[1], [256, 2], [1, 128]],
                )
                mskb = bass.AP(
                    tensor=msk_t.tensor,
                    offset=msk_t.offset,
                    ap=[msk_t.ap[0], [0, 2], [0, 2], msk_t.ap[1]],
                )
                nc.gpsimd.tensor_mul(out=pexv, in0=pexv, in1=mskb)
                pexs[(bi, pr)] = pex
            return th

        def stage_b_thunks(pair, vs, pexs):
            """Deferred sums + AV + normalize + output projection for `pair`.

            Returns (sums_thunks[6], av_thunks[6], y_thunks[2]); issued during
            the NEXT pair's slot so all cross-engine deps have slack.
            """
            attTs = {}
            nbrs = {}

            def mk_sums(bi, pr):
                def th():
                    pex = pexs[(bi, pr)]
                    nb = psa.tile([128, 256], F32, tag="work",
                                  name=f"nb_{pair}_{bi}_{pr}")
                    for two in range(2):
                        lo = two * 64
                        nc.tensor.matmul(
                            nb[lo : lo + 64, 0:256],
                            lhsT=ones_sq,
                            rhs=pex[:, two, 0:256],
                            start=True,
                            stop=False,
                            skip_group_check=True,
                        )
                        nc.tensor.matmul(
                            nb[lo : lo + 64, 128:256],
                            lhsT=ones_sq,
                            rhs=pex[:, two, 256:384],
                            start=False,
                            stop=True,
                            skip_group_check=True,
                        )
                    nbr = sb.tile([128, 256], F32, tag="nbr", bufs=8,
                                  name=f"nbr_{pair}_{bi}_{pr}")
                    nc.vector.reciprocal_approx_fast(out=nbr, in_=nb)
                    nbrs[(bi, pr)] = nbr
                return th

            def mk_av(bi, pr):
                def th():
                    if pr == 0:
                        attTs[bi] = sb.tile(
                            [128, 3, 256], DT, tag="attT", bufs=4,
                            name=f"attT_{pair}_{bi}",
                        )
                    attT = attTs[bi]
                    pex = pexs[(bi, pr)]
                    v = vs[bi]
                    av = psa.tile([128, 256], F32, tag="work",
                                  name=f"av_{pair}_{bi}_{pr}")
                    for two in range(2):
                        h = 2 * pr + two
                        lo = two * 64
                        hs = slice(h * 64, h * 64 + 64)
                        nc.tensor.matmul(
                            av[lo : lo + 64, 0:256],
                            lhsT=v[:, 0, hs],
                            rhs=pex[:, two, 0:256],
                            start=True,
                            stop=False,
                            skip_group_check=True,
                        )
                        nc.tensor.matmul(
                            av[lo : lo + 64, 128:256],
                            lhsT=v[:, 1, hs],
                            rhs=pex[:, two, 256:384],
                            start=False,
                            stop=True,
                            skip_group_check=True,
                        )
                    # fused normalize + PSUM->SBUF move
                    nc.vector.tensor_mul(out=attT[:, pr, :], in0=av,
                                         in1=nbrs[(bi, pr)])
                return th

            def mk_y(bi):
                def th():
                    attT = attTs[bi]
                    for m in range(2):
                        y_ps = psa.tile([128, C], F32, tag="work",
                                        name=f"y_{pair}_{bi}_{m}")
                        for k in range(3):
                            nc.tensor.matmul(
                                y_ps,
                                lhsT=attT[:, k, bass.ts(m, 128)],
                                rhs=wp_t[:, k, :],
                                start=(k == 0),
                                stop=(k == 2),
                            )
                        ysb = sb.tile([128, C], DT, tag="ysb", bufs=4,
                                      name=f"ysb_{pair}_{bi}_{m}")
                        nc.vector.tensor_add(out=ysb, in0=y_ps, in1=bb_t)
                        nc.sync.dma_start(
                            out=y[2 * pair + bi, bass.ts(m, 128), :], in_=ysb
                        )
                return th

            sums = [mk_sums(bi, pr) for bi in range(2) for pr in range(3)]
            avs = [mk_av(bi, pr) for bi in range(2) for pr in range(3)]
            ys = [mk_y(bi) for bi in range(2)]
            return sums, avs, ys

        prev_b = None
        for pair in range(npair):
            if pair + 2 < npair:
                xt_dma(pair + 2)
            vs, qt, kt = stage_proj(pair)
            pexs = {}
            score_thunks = [
                mk_score_chain(pair, qt, kt, pexs, bi, pr)
                for bi in range(2) for pr in range(3)
            ]
            cur_b = stage_b_thunks(pair, vs, pexs)
            # interleave: scores of `pair` spread between the previous pair's
            # sums so ACT's exp drain keeps pace with st-ring allocation
            if prev_b is None:
                for th in score_thunks:
                    th()
            else:
                sums, avs, ys = prev_b
                for i in range(6):
                    score_thunks[i]()
                    sums[i]()
                if pair == npair - 1:
                    # final slot: pull the last pair's own sums/AV/y into the
                    # stream so the tail's DVE chain overlaps PE work
                    sums7, avs7, ys7 = cur_b
                    for th in avs:
                        th()
                    for i in range(3):
                        sums7[i]()
                    ys[0]()
                    for i in range(3, 6):
                        sums7[i]()
                    ys[1]()
                    for i in range(3):
                        avs7[i]()
                    ys7[0]()
                    for i in range(3, 6):
                        avs7[i]()
                    ys7[1]()
                    cur_b = None
                else:
                    for th in avs:
                        th()
                    for th in ys:
                        th()
            prev_b = cur_b

        if prev_b is not None:
            sums, avs, ys = prev_b
            for bi in range(2):
                for i in range(3 * bi, 3 * bi + 3):
                    sums[i]()
                for i in range(3 * bi, 3 * bi + 3):
                    avs[i]()
                ys[bi]()
    nc.compile()
    return nc


def pack_inputs(x, Wq, Wk, Wv, Wp, bp):
    """Host-side packing. Returns (common weight map, per-core xT shards)."""
    from einops import rearrange

    x = np.asarray(x, np.float32)
    Wq = np.asarray(Wq, np.float32)
    Wk = np.asarray(Wk, np.float32)
    Wv = np.asarray(Wv, np.float32)
    Wp = np.asarray(Wp, np.float32)
    bp = np.asarray(bp, np.float32)

    scale = 1.0 / np.sqrt(np.float32(HS))
    wq_h = rearrange(Wq * scale, "(p two) (k c) d -> c k p (two d)", two=2, k=3)
    wk_h = rearrange(Wk, "(p two) (k c) d -> c k p (two d)", two=2, k=3)
    wv_h = rearrange(Wv, "h (k c) d -> c k (h d)", k=3)
    wp_h = rearrange(Wp, "c2 (k c1) -> c1 k c2", k=3)

    # multiplicative causal mask for a diagonal [128,128] block of the
    # TRANSPOSED scores st[s, tq]: keep tq >= s, i.e. 1 if j >= i else 0
    msk_h = np.triu(np.ones((128, 128), np.float32))
    bb_h = np.tile(bp[None, :], (128, 1)).astype(np.float32)

    common = {
        "wq8": np.ascontiguousarray(wq_h[:, 0:2] * FP8S).astype(NPF8),
        "wk8": np.ascontiguousarray(wk_h[:, 0:2] * FP8S).astype(NPF8),
        "wq2": np.ascontiguousarray(wq_h[:, 2] * FP8S).astype(NPDT),
        "wk2": np.ascontiguousarray(wk_h[:, 2] * FP8S).astype(NPDT),
        "wv": np.ascontiguousarray(wv_h).astype(NPDT),
        "wp": np.ascontiguousarray(wp_h).astype(NPDT),
        "msk": msk_h.astype(NPDT),
        "bb": bb_h,
    }
    shards = []
    for c in range(NCORES):
        xs = x[c * BPC : (c + 1) * BPC]  # [BPC, T, C]
        # paired layout: [pair, kc, c_local, b'*T + t]
        xp = xs.reshape(BPC // 2, 2, T, C).transpose(0, 3, 1, 2)  # [pair, C, 2, T]
        xTs = xp.reshape(BPC // 2, 3, 128, 2 * T)
        shards.append((
            np.ascontiguousarray(xTs).astype(NPDT),
            np.ascontiguousarray(xTs[:, 0:2].transpose(0, 2, 1, 3)).astype(NPF8),
        ))
    return common, shards


_NC_CACHE = {}


def _get_nc(n_batch: int = BPC) -> bass.Bass:
    if n_batch not in _NC_CACHE:
        _NC_CACHE[n_batch] = build(n_batch)
    return _NC_CACHE[n_batch]


def kernel(x, Wq, Wk, Wv, Wp, bp):
    common, shards = pack_inputs(x, Wq, Wk, Wv, Wp, bp)
    nc = _get_nc()
    in_maps = [
        {**common, "xT": shards[c][0], "xT8": shards[c][1]}
        for c in range(NCORES)
    ]
    res = run_bass_kernel_spmd(nc, in_maps, list(range(NCORES))).results
    y = np.concatenate([res[c]["y"] for c in range(NCORES)], axis=0)
    return np.ascontiguousarray(y.astype(np.float32))


# revision 32
# speedup vs baseline: 1.0121x; 1.0069x over previous
"""Multi-head causal self-attention (B=128, T=256, C=384, H=6, HS=64) for 8 TRN2 cores.

Strategy: pure data-parallel over batch (16 batch elements per core), weights
replicated, no collectives. Per batch-pair (2 elems, processed jointly):

  - x^T (pre-transposed on host, [C, 2T]) is the shared rhs/lhsT for projections
  - Q^T, K^T computed per head-pair as [128(2*d), 512(2*t)] PSUM tiles
  - V computed in natural [t, (h d)] layout
  - scores = K^T-slices.T @ Q^T with causal block-skipping, packed
    [s x tq(0:256) | s+128 x tq(128:256)] per head
  - softmax without max-subtraction (scores bounded for this distribution):
    exp on ACT, multiplicative causal mask on GpSimd.
  - row sums via PE matmul with a ones[128,64] stationary: the sums come out
    of the PE ALREADY BROADCAST across partitions, head A in partitions 0:64
    and head B in 64:128 (tile_position col-base 64). One DVE reciprocal per
    chain gives the normalizer tile nbr [128, 256] directly -- no separate
    broadcast matmul, no serialized [1,512] recip/cast round trip.
  - AV runs on the UNNORMALIZED exp'd scores (2 heads packed per PSUM tile,
    head B at partition base 64); normalization happens after AV as a single
    fused DVE multiply per chain: attT(bf16,SBUF) = av(PSUM) * nbr, which also
    replaces the separate PSUM->SBUF attT copy.
  - y = att^T.T @ Wp^T + bp, bias fused into the PSUM->SBUF move on DVE,
    y stored/DMA'd as bf16.

Scheduling, all aimed at keeping the PE HAM clock gate at 8/8 (2.4 GHz) by
never letting the PE idle:
  - software pipeline one pair deep: PE slot for pair N is
    [proj N][scores N (6 chains) interleaved with sums N-1][AV N-1][y N-1];
    every cross-engine dependency (ACT exp, GpSimd mask, DVE recip/norm) has
    microseconds of slack.
  - the score chains are SPREAD between sums thunks because ACT's exp drain
    (463ns/tile) is slower than the PE's score-tile fill (320ns/tile): a
    burst of 12 score tiles would exhaust any PSUM ring and stall the PE.
  - PSUM is bank-granular (8 banks): tag "st" gets a 3-bank ring, tag "work"
    (everything else) a 5-bank ring, so score-tile retirement (ACT) and
    work-tile retirement (DVE) never couple.
  - input DMAs are split per-k-chunk across 4 queues (sync/gpsimd/scalar/
    vector) so the first projection matmul can start as soon as the engine
    preambles end, instead of serializing 300KB+ behind one queue.

Matmul operands in bf16 (fp32 PSUM accumulation), softmax stats in fp32.
"""

import numpy as np
import ml_dtypes
from contextlib import ExitStack

import concourse.bass as bass
import concourse.bacc as bacc
import concourse.mybir as mybir
import concourse.tile as tile
from concourse.bass_utils import run_bass_kernel_spmd

B, T, C, H, HS = 128, 256, 384, 6, 64
NCORES = 8
BPC = B // NCORES  # batch elements per core

F32 = mybir.dt.float32
DT = mybir.dt.bfloat16
F8 = mybir.dt.float8e4
NPDT = ml_dtypes.bfloat16
NPF8 = ml_dtypes.float8_e4m3
DR = mybir.MatmulPerfMode.DoubleRow

EXP = mybir.ActivationFunctionType.Exp

# fp8 weight pre-scale: Wq/Wk values (~2.5e-3 std) sit in e4m3's subnormal
# range, so both are scaled by FP8S on the host; the combined 1/FP8S^2 comes
# out for free via the exp activation's scalar `scale`.
FP8S = 64.0


def build(n_batch: int = BPC) -> bass.Bass:
    assert n_batch % 2 == 0
    npair = n_batch // 2
    nc = bacc.Bacc("TRN2", target_bir_lowering=False, debug=False)

    xT = nc.dram_tensor("xT", [npair, 3, 128, 2 * T], DT, kind="ExternalInput").ap()
    xT8 = nc.dram_tensor("xT8", [npair, 128, 2, 2 * T], F8, kind="ExternalInput").ap()
    wq8 = nc.dram_tensor("wq8", [128, 2, 3, 128], F8, kind="ExternalInput").ap()
    wk8 = nc.dram_tensor("wk8", [128, 2, 3, 128], F8, kind="ExternalInput").ap()
    wq2 = nc.dram_tensor("wq2", [128, 3, 128], DT, kind="ExternalInput").ap()
    wk2 = nc.dram_tensor("wk2", [128, 3, 128], DT, kind="ExternalInput").ap()
    wv = nc.dram_tensor("wv", [128, 3, C], DT, kind="ExternalInput").ap()
    wp = nc.dram_tensor("wp", [128, 3, C], DT, kind="ExternalInput").ap()
    msk = nc.dram_tensor("msk", [128, 128], DT, kind="ExternalInput").ap()
    bb = nc.dram_tensor("bb", [128, C], F32, kind="ExternalInput").ap()
    y = nc.dram_tensor("y", [n_batch, T, C], DT, kind="ExternalOutput").ap()

    with tile.TileContext(nc) as tc, ExitStack() as ctx:
        const = ctx.enter_context(tc.tile_pool(name="const", bufs=1))
        sb = ctx.enter_context(tc.tile_pool(name="sb", bufs=2))
        psa = ctx.enter_context(tc.tile_pool(name="psa", bufs=5, space="PSUM"))

        # --- weight / first-x DMAs, split per k-chunk across 3 queues so
        # the pieces the first projection matmuls need land first ---
        wq8_t = const.tile([128, 2, 3, 128], F8)
        wk8_t = const.tile([128, 2, 3, 128], F8)
        wq2_t = const.tile([128, 3, 128], DT)
        wk2_t = const.tile([128, 3, 128], DT)
        wv_t = const.tile([128, 3, C], DT)
        wp_t = const.tile([128, 3, C], DT)
        msk_t = const.tile([128, 128], DT)
        bb_t = const.tile([128, C], F32)

        xts = {}

        def xt_dma(pair):
            xt = sb.tile([128, 3, 2 * T], DT, tag="xt", bufs=4, name=f"xt_{pair}")
            xt8 = sb.tile([128, 2, 2 * T], F8, tag="xt8", bufs=4,
                          name=f"xt8_{pair}")
            nc.sync.dma_start(out=xt[:, 0, :], in_=xT[pair, 0])
            nc.gpsimd.dma_start(out=xt[:, 1, :], in_=xT[pair, 1])
            nc.scalar.dma_start(out=xt[:, 2, :], in_=xT[pair, 2])
            nc.sync.dma_start(out=xt8, in_=xT8[pair])
            xts[pair] = (xt, xt8)

        # HAM warm-up + early-queue boilerplate on gpsimd (earliest preamble)
        warm_in = const.tile([128, 512], DT)
        nc.gpsimd.memset(warm_in, 0.0)
        ones_sq = const.tile([128, 64], DT)
        nc.gpsimd.memset(ones_sq, 1.0)

        # pair 0 / 1 x prefetch interleaved with the weight chunks; ordered
        # per-queue so arrival matches first-use order of the matmul stream
        xt0 = sb.tile([128, 3, 2 * T], DT, tag="xt", bufs=4, name="xt_0")
        xt08 = sb.tile([128, 2, 2 * T], F8, tag="xt8", bufs=4, name="xt8_0")
        xt1 = sb.tile([128, 3, 2 * T], DT, tag="xt", bufs=4, name="xt_1")
        xt18 = sb.tile([128, 2, 2 * T], F8, tag="xt8", bufs=4, name="xt8_1")
        xts[0] = (xt0, xt08)
        xts[1] = (xt1, xt18)
        nc.scalar.dma_start(out=wq8_t, in_=wq8)
        nc.sync.dma_start(out=xt08, in_=xT8[0])
        nc.sync.dma_start(out=wk8_t, in_=wk8)
        nc.scalar.dma_start(out=xt0[:, 2, :], in_=xT[0, 2])
        nc.gpsimd.dma_start(out=wq2_t, in_=wq2)
        nc.scalar.dma_start(out=wk2_t, in_=wk2)
        nc.sync.dma_start(out=xt0[:, 0, :], in_=xT[0, 0])
        nc.gpsimd.dma_start(out=xt0[:, 1, :], in_=xT[0, 1])
        nc.gpsimd.dma_start(out=wv_t[:, 0], in_=wv[:, 0])
        nc.scalar.dma_start(out=wv_t[:, 2], in_=wv[:, 2])
        nc.sync.dma_start(out=wv_t[:, 1], in_=wv[:, 1])
        nc.gpsimd.dma_start(out=msk_t, in_=msk)
        nc.scalar.dma_start(out=xt1[:, 2, :], in_=xT[1, 2])
        nc.gpsimd.dma_start(out=xt1[:, 1, :], in_=xT[1, 1])
        nc.sync.dma_start(out=xt1[:, 0, :], in_=xT[1, 0])
        nc.sync.dma_start(out=xt18, in_=xT8[1])
        nc.gpsimd.dma_start(out=bb_t, in_=bb)
        nc.gpsimd.dma_start(out=wp_t, in_=wp)

        warm_ps = psa.tile([128, 512], F32, tag="work")
        for _ in range(12):
            nc.tensor.matmul(
                warm_ps, lhsT=warm_in[:, 0:128], rhs=warm_in,
                start=True, stop=True,
            )

        def filler(n):
            # pair-0 only: dummy matmuls bridging initial DMA-wait gaps so
            # the HAM busy-window keeps accumulating and un-throttles early
            fps = psa.tile([128, 384], F32, tag="st", bufs=3, name="filler")
            for _ in range(n):
                nc.tensor.matmul(
                    fps, lhsT=warm_in[:, 0:128], rhs=warm_in[:, 0:384],
                    start=True, stop=True,
                )

        def stage_proj(pair):
            """QKV projections for `pair` (qt/kt copies: qt->ACT, kt/v->DVE).

            Q/K contract k-chunks 0+1 in one fp8 DoubleRow matmul (weights
            pre-scaled by FP8S on host) and k-chunk 2 in bf16.
            """
            xt, xt8 = xts.pop(pair)
            qt = sb.tile([128, 3, 2 * T], DT, tag="qt", bufs=3)
            kt = sb.tile([128, 3, 2 * T], DT, tag="kt", bufs=3)
            for p in range(3):
                qt_ps = psa.tile([128, 2 * T], F32, tag="work")
                nc.tensor.matmul(
                    qt_ps,
                    lhsT=wq8_t[:, :, p, :],
                    rhs=xt8,
                    start=True,
                    stop=False,
                    perf_mode=DR,
                )
                nc.tensor.matmul(
                    qt_ps,
                    lhsT=wq2_t[:, p, :],
                    rhs=xt[:, 2, :],
                    start=False,
                    stop=True,
                )
                nc.scalar.copy(out=qt[:, p, :], in_=qt_ps)
                kt_ps = psa.tile([128, 2 * T], F32, tag="work")
                nc.tensor.matmul(
                    kt_ps,
                    lhsT=wk8_t[:, :, p, :],
                    rhs=xt8,
                    start=True,
                    stop=False,
                    perf_mode=DR,
                )
                nc.tensor.matmul(
                    kt_ps,
                    lhsT=wk2_t[:, p, :],
                    rhs=xt[:, 2, :],
                    start=False,
                    stop=True,
                )
                nc.vector.tensor_copy(out=kt[:, p, :], in_=kt_ps)

            vs = []
            for bi in range(2):
                v = sb.tile([128, 2, C], DT, tag="v", bufs=6, name=f"v_{pair}_{bi}")
                for m in range(2):
                    v_ps = psa.tile([128, C], F32, tag="work")
                    for k in range(3):
                        nc.tensor.matmul(
                            v_ps,
                            lhsT=xt[:, k, bi * T + m * 128 : bi * T + (m + 1) * 128],
                            rhs=wv_t[:, k, :],
                            start=(k == 0),
                            stop=(k == 2),
                        )
                    nc.vector.tensor_copy(out=v[:, m, :], in_=v_ps)
                vs.append(v)
            return vs, qt, kt

        def mk_score_chain(pair, qt, kt, pexs, bi, pr):
            """One (bi, pr) head-pair: 4 score MMs + 2 exps + 1 mask."""
            def th():
                pex = sb.tile(
                    [128, 2, 384], DT, tag="pex", bufs=14,
                    name=f"pex_{pair}_{bi}_{pr}",
                )
                for two in range(2):
                    lo = two * 64
                    qh = qt[lo : lo + 64, pr, bi * T : (bi + 1) * T]
                    kh = kt[lo : lo + 64, pr, bi * T : (bi + 1) * T]
                    st = psa.tile([128, 384], F32, tag="st", bufs=3)
                    nc.tensor.matmul(
                        st[:, 0:256],
                        lhsT=kh[:, 0:128],
                        rhs=qh,
                        start=True,
                        stop=True,
                    )
                    nc.tensor.matmul(
                        st[:, 256:384],
                        lhsT=kh[:, 128:256],
                        rhs=qh[:, 128:256],
                        start=True,
                        stop=True,
                    )
                    nc.scalar.activation(out=pex[:, two, :], in_=st, func=EXP,
                                         scale=1.0 / (FP8S * FP8S))
                # multiplicative causal mask (keep tq >= s) on the four
                # triangular blocks of the packed 2-head pex in ONE op:
                # dims [p][head][block in {0:128, 256:384}][j]
                pexv = bass.AP(
                    tensor=pex.tensor,
                    offset=pex.offset,
                    ap=[pex.ap[0], pex.ap[1], [256, 2], [1, 128]],
                )
                mskb = bass.AP(
                    tensor=msk_t.tensor,
                    offset=msk_t.offset,
                    ap=[msk_t.ap[0], [0, 2], [0, 2], msk_t.ap[1]],
                )
                nc.gpsimd.tensor_mul(out=pexv, in0=pexv, in1=mskb)
                pexs[(bi, pr)] = pex
            return th

        def stage_b_thunks(pair, vs, pexs):
            """Deferred sums + AV + normalize + output projection for `pair`.

            Returns (sums_thunks[6], av_thunks[6], y_thunks[2]); issued during
            the NEXT pair's slot so all cross-engine deps have slack.
            """
            attTs = {}
            nbrs = {}

            def mk_sums(bi, pr):
                def th():
                    pex = pexs[(bi, pr)]
                    nb = psa.tile([128, 256], F32, tag="work",
                                  name=f"nb_{pair}_{bi}_{pr}")
                    for two in range(2):
                        lo = two * 64
                        nc.tensor.matmul(
                            nb[lo : lo + 64, 0:256],
                            lhsT=ones_sq,
                            rhs=pex[:, two, 0:256],
                            start=True,
                            stop=False,
                            skip_group_check=True,
                        )
                        nc.tensor.matmul(
                            nb[lo : lo + 64, 128:256],
                            lhsT=ones_sq,
                            rhs=pex[:, two, 256:384],
                            start=False,
                            stop=True,
                            skip_group_check=True,
                        )
                    nbr = sb.tile([128, 256], F32, tag="nbr", bufs=8,
                                  name=f"nbr_{pair}_{bi}_{pr}")
                    nc.vector.reciprocal_approx_fast(out=nbr, in_=nb)
                    nbrs[(bi, pr)] = nbr
                return th

            def mk_av(bi, pr):
                def th():
                    if pr == 0:
                        attTs[bi] = sb.tile(
                            [128, 3, 256], DT, tag="attT", bufs=4,
                            name=f"attT_{pair}_{bi}",
                        )
                    attT = attTs[bi]
                    pex = pexs[(bi, pr)]
                    v = vs[bi]
                    av = psa.tile([128, 256], F32, tag="work",
                                  name=f"av_{pair}_{bi}_{pr}")
                    for two in range(2):
                        h = 2 * pr + two
                        lo = two * 64
                        hs = slice(h * 64, h * 64 + 64)
                        nc.tensor.matmul(
                            av[lo : lo + 64, 0:256],
                            lhsT=v[:, 0, hs],
                            rhs=pex[:, two, 0:256],
                            start=True,
                            stop=False,
                            skip_group_check=True,
                        )
                        nc.tensor.matmul(
                            av[lo : lo + 64, 128:256],
                            lhsT=v[:, 1, hs],
                            rhs=pex[:, two, 256:384],
                            start=False,
                            stop=True,
                            skip_group_check=True,
                        )
                    # fused normalize + PSUM->SBUF move
                    nc.vector.tensor_mul(out=attT[:, pr, :], in0=av,
                                         in1=nbrs[(bi, pr)])
                return th

            def mk_y(bi):
                def th():
                    attT = attTs[bi]
                    for m in range(2):
                        y_ps = psa.tile([128, C], F32, tag="work",
                                        name=f"y_{pair}_{bi}_{m}")
                        for k in range(3):
                            nc.tensor.matmul(
                                y_ps,
                                lhsT=attT[:, k, bass.ts(m, 128)],
                                rhs=wp_t[:, k, :],
                                start=(k == 0),
                                stop=(k == 2),
                            )
                        ysb = sb.tile([128, C], DT, tag="ysb", bufs=4,
                                      name=f"ysb_{pair}_{bi}_{m}")
                        nc.vector.tensor_add(out=ysb, in0=y_ps, in1=bb_t)
                        nc.sync.dma_start(
                            out=y[2 * pair + bi, bass.ts(m, 128), :], in_=ysb
                        )
                return th

            sums = [mk_sums(bi, pr) for bi in range(2) for pr in range(3)]
            avs = [mk_av(bi, pr) for bi in range(2) for pr in range(3)]
            ys = [mk_y(bi) for bi in range(2)]
            return sums, avs, ys

        prev_b = None
        for pair in range(npair):
            if pair + 2 < npair:
                xt_dma(pair + 2)
            vs, qt, kt = stage_proj(pair)
            pexs = {}
            score_thunks = [
                mk_score_chain(pair, qt, kt, pexs, bi, pr)
                for bi in range(2) for pr in range(3)
            ]
            cur_b = stage_b_thunks(pair, vs, pexs)
            # interleave: scores of `pair` spread between the previous pair's
            # sums so ACT's exp drain keeps pace with st-ring allocation
            if prev_b is None:
                for th in score_thunks:
                    th()
            else:
                sums, avs, ys = prev_b
                for i in range(6):
                    score_thunks[i]()
                    sums[i]()
                if pair == npair - 1:
                    # final slot: pull the last pair's own sums/AV/y into the
                    # stream so the tail's DVE chain overlaps PE work
                    sums7, avs7, ys7 = cur_b
                    for th in avs:
                        th()
                    for i in range(3):
                        sums7[i]()
                    ys[0]()
                    for i in range(3, 6):
                        sums7[i]()
                    ys[1]()
                    for i in range(3):
                        avs7[i]()
                    ys7[0]()
                    for i in range(3, 6):
                        avs7[i]()
                    ys7[1]()
                    cur_b = None
                else:
                    for th in avs:
                        th()
                    for th in ys:
                        th()
            prev_b = cur_b

        if prev_b is not None:
            sums, avs, ys = prev_b
            for bi in range(2):
                for i in range(3 * bi, 3 * bi + 3):
                    sums[i]()
                for i in range(3 * bi, 3 * bi + 3):
                    avs[i]()
                ys[bi]()
    nc.compile()
    return nc


def pack_inputs(x, Wq, Wk, Wv, Wp, bp):
    """Host-side packing. Returns (common weight map, per-core xT shards)."""
    from einops import rearrange

    x = np.asarray(x, np.float32)
    Wq = np.asarray(Wq, np.float32)
    Wk = np.asarray(Wk, np.float32)
    Wv = np.asarray(Wv, np.float32)
    Wp = np.asarray(Wp, np.float32)
    bp = np.asarray(bp, np.float32)

    scale = 1.0 / np.sqrt(np.float32(HS))
    wq_h = rearrange(Wq * scale, "(p two) (k c) d -> c k p (two d)", two=2, k=3)
    wk_h = rearrange(Wk, "(p two) (k c) d -> c k p (two d)", two=2, k=3)
    wv_h = rearrange(Wv, "h (k c) d -> c k (h d)", k=3)
    wp_h = rearrange(Wp, "c2 (k c1) -> c1 k c2", k=3)

    # multiplicative causal mask for a diagonal [128,128] block of the
    # TRANSPOSED scores st[s, tq]: keep tq >= s, i.e. 1 if j >= i else 0
    msk_h = np.triu(np.ones((128, 128), np.float32))
    bb_h = np.tile(bp[None, :], (128, 1)).astype(np.float32)

    common = {
        "wq8": np.ascontiguousarray(wq_h[:, 0:2] * FP8S).astype(NPF8),
        "wk8": np.ascontiguousarray(wk_h[:, 0:2] * FP8S).astype(NPF8),
        "wq2": np.ascontiguousarray(wq_h[:, 2] * FP8S).astype(NPDT),
        "wk2": np.ascontiguousarray(wk_h[:, 2] * FP8S).astype(NPDT),
        "wv": np.ascontiguousarray(wv_h).astype(NPDT),
        "wp": np.ascontiguousarray(wp_h).astype(NPDT),
        "msk": msk_h.astype(NPDT),
        "bb": bb_h,
    }
    shards = []
    for c in range(NCORES):
        xs = x[c * BPC : (c + 1) * BPC]  # [BPC, T, C]
        # paired layout: [pair, kc, c_local, b'*T + t]
        xp = xs.reshape(BPC // 2, 2, T, C).transpose(0, 3, 1, 2)  # [pair, C, 2, T]
        xTs = xp.reshape(BPC // 2, 3, 128, 2 * T)
        shards.append((
            np.ascontiguousarray(xTs).astype(NPDT),
            np.ascontiguousarray(xTs[:, 0:2].transpose(0, 2, 1, 3)).astype(NPF8),
        ))
    return common, shards


_NC_CACHE = {}


def _get_nc(n_batch: int = BPC) -> bass.Bass:
    if n_batch not in _NC_CACHE:
        _NC_CACHE[n_batch] = build(n_batch)
    return _NC_CACHE[n_batch]


def kernel(x, Wq, Wk, Wv, Wp, bp):
    common, shards = pack_inputs(x, Wq, Wk, Wv, Wp, bp)
    nc = _get_nc()
    in_maps = [
        {**common, "xT": shards[c][0], "xT8": shards[c][1]}
        for c in range(NCORES)
    ]
    res = run_bass_kernel_spmd(nc, in_maps, list(range(NCORES))).results
    y = np.concatenate([res[c]["y"] for c in range(NCORES)], axis=0)
    return np.ascontiguousarray(y.astype(np.float32))
